# revision 42
# baseline (speedup 1.0000x reference)
"""Trainium2 Bass kernel for nn_MinimalLoss (YOLO-style detection loss).

Strategy (data-parallel over 8 NeuronCores, 4 batches each):
  The only parts of `predictions` [B, HW, 85] that matter are:
    * column 4 (conf logit) of every cell  -> sum of softplus(x)
      ( -ln(1-sigmoid(x)) = ln(1+e^x) ), computed with Exp+Ln activations
      (both live in the same act-func table set -> one table load total).
    * the <=200 rows per core addressed by targets -> gathered via
      indirect DMA; xy/wh/cls/conf-correction terms computed on-chip.

  The strided conf-channel read (one 4-byte element per 340B row) is the
  problem's bottleneck: 102,400 scattered 64B HBM bursts per core at the
  measured ~1.6-1.8 scattered-reads/ns/core ceiling -> a ~61-64us DMA
  window that everything else (per-target math, dedup, reductions) is
  overlapped under.  Per-core partials (5 stats via a PSUM matmul + the
  raw conf accumulator) are combined on host.
"""
import numpy as np

import concourse.bass as bass
import concourse.mybir as mybir
import concourse.tile as tile
from concourse.bass import IndirectOffsetOnAxis
from concourse.masks import make_identity

F32 = mybir.dt.float32
I32 = mybir.dt.int32
AF = mybir.ActivationFunctionType
ALU = mybir.AluOpType
AX = mybir.AxisListType

B, HWC, C, T = 32, 25600, 80, 50          # full problem
H = W = 160
NCORES = 8
BL = B // NCORES                          # 4 batches per core
ROWS = BL * HWC                           # 102400 prediction rows per core
NT = BL * T                               # 200 targets per core
HALF = NT // 2                            # 100 targets per half (2 batches)
MAGIC = float(np.float32(2 ** 23))

# conf transport: all via the sync HWDGE ring, one 4B descriptor/packet
# per cell, 16 chunks.  The binding resource is the HBM scattered-read
# rate (~1.7-1.8 reads/ns/core; every cell is its own 64B burst), so the
# ~61-64us window is irreducible for this layout.  Measured dead ends:
#   * a second HWDGE queue (scalar): same total rate, and descriptor
#     generation blocks the issuing engine's sequencer (ring
#     backpressure), starving all activations;
#   * SWDGE (gpsimd Q7 gen, ~0.74ns/desc): adding it on top of HWDGE
#     yields only the same shared ~1.8 reads/ns, minus ring-oscillation
#     and per-packet round-robin losses — measured net-negative;
#   * 344B "pair" descriptors (2 cells/run): 172B/cell of engine bus and
#     3 HBM bursts/cell cap them below singles.
import os as _os
SW_COLS = int(_os.environ.get("SW_COLS", "0"))    # experimental SWDGE share
HW_COLS = 800 - SW_COLS
NCH_HW = 16
assert HW_COLS % NCH_HW == 0
if SW_COLS == 0:
    SW_CHUNKS = []
elif SW_COLS >= 64:
    _body = SW_COLS - 32
    _n = max(1, round(_body / 42))
    SW_CHUNKS = [16, 16] + [_body // _n + (1 if i < _body % _n else 0)
                            for i in range(_n)]
else:
    SW_CHUNKS = [SW_COLS]
assert sum(SW_CHUNKS) == SW_COLS
NJOBS = len(SW_CHUNKS) + NCH_HW


def _floor(nc, sb, dst, src, n):
    """dst = floor(src) for 0 <= src < 2^22, exact (round-to-nearest fixup)."""
    r = sb.tile([n, 1], F32, tag="fl_r")
    adj = sb.tile([n, 1], F32, tag="fl_a")
    nc.vector.tensor_scalar_add(r[:], src, MAGIC)
    nc.vector.tensor_scalar_add(r[:], r[:], -MAGIC)
    nc.vector.tensor_tensor(out=adj[:], in0=r[:], in1=src, op=ALU.is_gt)
    nc.vector.tensor_tensor(out=dst, in0=r[:], in1=adj[:], op=ALU.subtract)


def _split_multi_waits(nc):
    """Walrus codegen accepts at most ONE sync wait per instruction; hoist
    extras onto standalone EventSemaphore (wait) ops on the same engine."""
    n = 0
    for func in nc.m.functions:
        for block in func.blocks:
            out = []
            for inst in block.instructions:
                si = inst.sync_info
                if si is not None and si.on_wait and len(si.on_wait) > 1:
                    waits = list(si.on_wait)
                    for w in waits[:-1]:
                        n += 1
                        nop = mybir.InstEventSemaphore(
                            name=f"{inst.name}_sw{n}", engine=inst.engine,
                            ins=[], outs=[])
                        nop.sync_info = mybir.SyncInfo(on_wait=[w], on_update=[])
                        out.append(nop)
                    inst.sync_info = mybir.SyncInfo(on_wait=[waits[-1]],
                                                    on_update=list(si.on_update))
                out.append(inst)
            if n:
                block.instructions[:] = out
    return n


def build_nc(split=True):
    nc = bass.Bass("TRN2", target_bir_lowering=False, debug=False)
    ngrp = 4 if not SW_CHUNKS else NJOBS  # ln/accum groups (4 chunks each)
    pred_d = nc.dram_tensor("predictions", [ROWS, 85], F32, kind="ExternalInput")
    tgt_d = nc.dram_tensor("targets", [NT, 5], F32, kind="ExternalInput")
    out_d = nc.dram_tensor("out", [8, 1], F32, kind="ExternalOutput")
    acc_d = nc.dram_tensor("acc", [128, ngrp], F32, kind="ExternalOutput")

    pred_ap = pred_d.ap()
    conf = pred_ap[:, 4:5].rearrange("(p j) o -> p (j o)", p=128)  # [128, 800]

    with tile.TileContext(nc) as tc:
        with tc.tile_pool(name="persist", bufs=1) as pp, \
             tc.tile_pool(name="conf", bufs=1) as cp, \
             tc.tile_pool(name="sb", bufs=2) as sb, \
             tc.tile_pool(name="ps", bufs=1, space="PSUM") as ps:

            # ---- target loads first on the scalar HWDGE ring (tiny).
            P = HALF
            tts = []
            for q in range(2):
                tt = sb.tile([P, 5], F32, tag="tt")
                nc.scalar.dma_start(out=tt[:], in_=tgt_d.ap()[q * P:(q + 1) * P, :])
                tts.append(tt)

            ctls = [None] * NJOBS

            # HWDGE conf chunks (unique all-live tiles so each DMA needs
            # <=1 sync wait — the DIRECT2D codegen limit).  The first SCQ
            # chunks go on the scalar HWDGE ring: two rings drain ~6%
            # faster than one (engines round-robin per packet), and a
            # small leading share only blocks the Scalar sequencer until
            # ~25us — long before any activation deadline.
            scq = int(_os.environ.get("SCQ", "3"))
            cw = HW_COLS // NCH_HW
            for k in range(NCH_HW):
                tl = cp.tile([128, cw], F32, tag=f"ci{k}")
                eng = nc.scalar if k < scq else nc.sync
                eng.dma_start(out=tl[:], in_=conf[:, k * cw:(k + 1) * cw])
                ctls[k] = tl

            # optional SWDGE conf chunks, cols [HW_COLS, 800)
            sw_off = [HW_COLS]
            for w in SW_CHUNKS:
                sw_off.append(sw_off[-1] + w)

            def emit_sw(i):
                w = SW_CHUNKS[i]
                tl = cp.tile([128, w], F32, tag=f"ci{NCH_HW + i}")
                nc.gpsimd.dma_start(out=tl[:], in_=conf[:, sw_off[i]:sw_off[i] + w])
                ctls[NCH_HW + i] = tl

            for i in range(min(2, len(SW_CHUNKS))):
                emit_sw(i)

            # constants (route matmul operands through DVE so each matmul
            # needs at most ONE sync wait — the S3_LW slot limit)
            ident_g = pp.tile([128, 128], F32)
            make_identity(nc, ident_g[:])
            ident = pp.tile([128, 128], F32)
            nc.vector.tensor_copy(out=ident[:], in_=ident_g[:])
            ones = pp.tile([128, 1], F32)
            nc.vector.memset(ones[:], 1.0)
            iotac = pp.tile([128, C], I32)
            nc.gpsimd.iota(iotac[:], pattern=[[1, C]], base=0, channel_multiplier=0)
            iotaf = pp.tile([128, C], F32)
            nc.vector.tensor_copy(out=iotaf[:], in_=iotac[:])
            iotap = pp.tile([128, 1], I32)
            nc.gpsimd.iota(iotap[:], pattern=[[1, 1]], base=0, channel_multiplier=1)
            pf128 = pp.tile([128, 1], F32)
            nc.vector.tensor_copy(out=pf128[:], in_=iotap[:])
            iotar = pp.tile([128, 128], I32)
            nc.gpsimd.iota(iotar[:], pattern=[[1, 128]], base=0, channel_multiplier=0)
            iotarf = pp.tile([128, 128], F32)
            nc.vector.tensor_copy(out=iotarf[:], in_=iotar[:])
            tri = pp.tile([128, 128], F32)  # tri[p, f] = 1.0 iff f < p
            nc.vector.tensor_tensor(out=tri[:], in0=pf128[:].to_broadcast([128, 128]),
                                    in1=iotarf[:], op=ALU.is_gt)

            acc = pp.tile([128, ngrp], F32)
            ex_all = pp.tile([128, HW_COLS], F32)

            # ---- per-target phase: two halves of 100 targets (2 whole
            # batches each).  Runs entirely under the conf-DMA window.
            stats_ps = ps.tile([5, 1], F32, space="PSUM")
            for q in range(2):
                tt = tts[q]

                xW = sb.tile([P, 1], F32, tag="xW")
                yH = sb.tile([P, 1], F32, tag="yH")
                nc.vector.tensor_scalar_mul(xW[:], tt[:, 1:2], float(W))
                nc.vector.tensor_scalar_mul(yH[:], tt[:, 2:3], float(H))
                gx = sb.tile([P, 1], F32, tag="gx")
                gy = sb.tile([P, 1], F32, tag="gy")
                _floor(nc, sb, gx[:], xW[:], P)
                _floor(nc, sb, gy[:], yH[:], P)

                # validity
                vf = sb.tile([P, 1], F32, tag="vf")
                tmp = sb.tile([P, 1], F32, tag="tmp")
                nc.vector.tensor_scalar(out=vf[:], in0=gx[:], scalar1=0.0, scalar2=None, op0=ALU.is_ge)
                nc.vector.tensor_scalar(out=tmp[:], in0=gx[:], scalar1=float(W), scalar2=None, op0=ALU.is_lt)
                nc.vector.tensor_tensor(out=vf[:], in0=vf[:], in1=tmp[:], op=ALU.mult)
                nc.vector.tensor_scalar(out=tmp[:], in0=gy[:], scalar1=0.0, scalar2=None, op0=ALU.is_ge)
                nc.vector.tensor_tensor(out=vf[:], in0=vf[:], in1=tmp[:], op=ALU.mult)
                nc.vector.tensor_scalar(out=tmp[:], in0=gy[:], scalar1=float(H), scalar2=None, op0=ALU.is_lt)
                nc.vector.tensor_tensor(out=vf[:], in0=vf[:], in1=tmp[:], op=ALU.mult)

                # cell + per-core row index
                gxi = sb.tile([P, 1], F32, tag="gxi")
                gyi = sb.tile([P, 1], F32, tag="gyi")
                nc.vector.tensor_scalar(out=gxi[:], in0=gx[:], scalar1=0.0, scalar2=float(W - 1),
                                        op0=ALU.max, op1=ALU.min)
                nc.vector.tensor_scalar(out=gyi[:], in0=gy[:], scalar1=0.0, scalar2=float(H - 1),
                                        op0=ALU.max, op1=ALU.min)
                cell = sb.tile([P, 1], F32, tag="cell")
                nc.vector.tensor_scalar_mul(cell[:], gyi[:], float(W))
                nc.vector.tensor_tensor(out=cell[:], in0=cell[:], in1=gxi[:], op=ALU.add)

                rowf = sb.tile([P, 1], F32, tag="rowf")
                # batch offset: (2q + (t>=50)) * HWC
                nc.vector.tensor_scalar(out=rowf[:], in0=pf128[:P, :], scalar1=float(T), scalar2=None,
                                        op0=ALU.is_ge)
                nc.vector.tensor_scalar(out=rowf[:], in0=rowf[:], scalar1=float(HWC),
                                        scalar2=float(2 * q * HWC), op0=ALU.mult, op1=ALU.add)
                nc.vector.tensor_tensor(out=rowf[:], in0=rowf[:], in1=cell[:], op=ALU.add)
                idx = sb.tile([P, 1], I32, tag="idx")
                nc.vector.tensor_copy(out=idx[:], in_=rowf[:])

                # dedup key: valid -> rowf ; invalid -> unique negative
                negk = sb.tile([P, 1], F32, tag="negk")
                nc.vector.tensor_scalar(out=negk[:], in0=pf128[:P, :], scalar1=-1.0,
                                        scalar2=-(1.0 + 100.0 * q), op0=ALU.mult, op1=ALU.add)
                key = sb.tile([P, 1], F32, tag="key")
                nc.vector.tensor_tensor(out=key[:], in0=rowf[:], in1=negk[:], op=ALU.subtract)
                nc.vector.tensor_tensor(out=key[:], in0=key[:], in1=vf[:], op=ALU.mult)
                nc.vector.tensor_tensor(out=key[:], in0=key[:], in1=negk[:], op=ALU.add)

                # gather prediction rows
                rows = sb.tile([P, 85], F32, tag="rows")
                nc.gpsimd.indirect_dma_start(
                    out=rows[:], out_offset=None, in_=pred_ap[:, :],
                    in_offset=IndirectOffsetOnAxis(ap=idx[:, :1], axis=0))

                # A  = ln(1+e^x)  = -ln(1-sigmoid(x))   (clamp never fires:
                # B  = ln(1+e^-x) = -ln(sigmoid(x))      |x| <~ 0.6 here)
                ep = sb.tile([P, 85], F32, tag="ep")
                nc.scalar.activation(out=ep[:], in_=rows[:], func=AF.Exp)
                en = sb.tile([P, 85], F32, tag="en")
                nc.scalar.activation(out=en[:], in_=rows[:], func=AF.Exp, scale=-1.0)
                At = sb.tile([P, 85], F32, tag="At")
                nc.scalar.activation(out=At[:], in_=ep[:], func=AF.Ln, bias=1.0)
                Bt = sb.tile([P, 85], F32, tag="Bt")
                nc.scalar.activation(out=Bt[:], in_=en[:], func=AF.Ln, bias=1.0)

                # per_cls = (1/C) * sum_c [ A + onehot*(B - A) ]
                oh = sb.tile([P, C], F32, tag="oh")
                nc.vector.tensor_tensor(out=oh[:], in0=iotaf[:P, :],
                                        in1=tt[:, 0:1].to_broadcast([P, C]), op=ALU.is_equal)
                dlt = sb.tile([P, C], F32, tag="dlt")
                nc.vector.tensor_tensor(out=dlt[:], in0=Bt[:, 5:85], in1=At[:, 5:85], op=ALU.subtract)
                nc.vector.tensor_tensor(out=dlt[:], in0=dlt[:], in1=oh[:], op=ALU.mult)
                nc.vector.tensor_tensor(out=dlt[:], in0=dlt[:], in1=At[:, 5:85], op=ALU.add)
                pcls = sb.tile([P, 1], F32, tag="pcls")
                nc.vector.reduce_sum(out=pcls[:], in_=dlt[:], axis=AX.X)
                nc.vector.tensor_scalar_mul(pcls[:], pcls[:], 1.0 / C)

                # conf correction term: ct = lnn - lnp = B[4] - A[4]
                ct = sb.tile([P, 1], F32, tag="ct")
                nc.vector.tensor_tensor(out=ct[:], in0=Bt[:, 4:5], in1=At[:, 4:5], op=ALU.subtract)

                # per_xy: sigmoid(x) = 1/(1+e^-x)
                sxy = sb.tile([P, 2], F32, tag="sxy")
                nc.vector.tensor_scalar_add(sxy[:], en[:, 0:2], 1.0)
                nc.vector.reciprocal(out=sxy[:], in_=sxy[:])
                txy = sb.tile([P, 2], F32, tag="txy")
                nc.vector.tensor_tensor(out=txy[:, 0:1], in0=xW[:], in1=gx[:], op=ALU.subtract)
                nc.vector.tensor_tensor(out=txy[:, 1:2], in0=yH[:], in1=gy[:], op=ALU.subtract)
                dxy = sb.tile([P, 2], F32, tag="dxy")
                nc.vector.tensor_tensor(out=dxy[:], in0=sxy[:], in1=txy[:], op=ALU.subtract)
                nc.vector.tensor_tensor(out=dxy[:], in0=dxy[:], in1=dxy[:], op=ALU.mult)
                pxy = sb.tile([P, 1], F32, tag="pxy")
                nc.vector.reduce_sum(out=pxy[:], in_=dxy[:], axis=AX.X)
                nc.vector.tensor_scalar_mul(pxy[:], pxy[:], 0.5)

                # per_wh: pred_wh = e^x (already have ep)
                twh = sb.tile([P, 2], F32, tag="twh")
                nc.vector.tensor_scalar_mul(twh[:, 0:1], tt[:, 3:4], float(W))
                nc.vector.tensor_scalar_mul(twh[:, 1:2], tt[:, 4:5], float(H))
                dwh = sb.tile([P, 2], F32, tag="dwh")
                nc.vector.tensor_tensor(out=dwh[:], in0=ep[:, 2:4], in1=twh[:], op=ALU.subtract)
                nc.vector.tensor_tensor(out=dwh[:], in0=dwh[:], in1=dwh[:], op=ALU.mult)
                pwh = sb.tile([P, 1], F32, tag="pwh1")
                nc.vector.reduce_sum(out=pwh[:], in_=dwh[:], axis=AX.X)
                nc.vector.tensor_scalar_mul(pwh[:], pwh[:], 0.5)

                # dedup: first-occurrence weight w
                keyT_ps = ps.tile([P, P], F32, space="PSUM", tag="keyT_ps")
                nc.tensor.transpose(out=keyT_ps[:], in_=key[:].to_broadcast([P, P]),
                                    identity=ident[:P, :P])
                keyT = sb.tile([P, P], F32, tag="keyT")
                nc.vector.tensor_copy(out=keyT[:], in_=keyT_ps[:])
                eq = sb.tile([P, P], F32, tag="eq")
                nc.vector.tensor_tensor(out=eq[:], in0=key[:].to_broadcast([P, P]),
                                        in1=keyT[:], op=ALU.is_equal)
                nc.vector.tensor_tensor(out=eq[:], in0=eq[:], in1=tri[:P, :P], op=ALU.mult)
                dup = sb.tile([P, 1], F32, tag="dup")
                nc.vector.reduce_max(out=dup[:], in_=eq[:], axis=AX.X)
                wfo = sb.tile([P, 1], F32, tag="wfo")
                nc.vector.tensor_scalar(out=wfo[:], in0=dup[:], scalar1=-1.0, scalar2=1.0,
                                        op0=ALU.mult, op1=ALU.add)
                nc.vector.tensor_tensor(out=wfo[:], in0=wfo[:], in1=vf[:], op=ALU.mult)

                # stats columns: vf*pxy, vf*pwh, vf*pcls, vf, w*ct
                stats = sb.tile([P, 5], F32, tag="stats")
                nc.vector.tensor_tensor(out=stats[:, 0:1], in0=pxy[:], in1=vf[:], op=ALU.mult)
                nc.vector.tensor_tensor(out=stats[:, 1:2], in0=pwh[:], in1=vf[:], op=ALU.mult)
                nc.vector.tensor_tensor(out=stats[:, 2:3], in0=pcls[:], in1=vf[:], op=ALU.mult)
                nc.vector.tensor_copy(out=stats[:, 3:4], in_=vf[:])
                nc.vector.tensor_tensor(out=stats[:, 4:5], in0=ct[:], in1=wfo[:], op=ALU.mult)

                nc.tensor.matmul(out=stats_ps[:], lhsT=stats[:], rhs=ones[:P, :],
                                 start=(q == 0), stop=(q == 1))

            # remaining SWDGE conf chunks (after the gathers in Pool order)
            for i in range(2, len(SW_CHUNKS)):
                emit_sw(i)

            # ---- conf channel: sum softplus(x) = ln(1 + e^x).
            # One Exp per chunk (pipelines with its DMA), but only one
            # Ln+accum per 4-chunk group: fewer Scalar ops and fewer
            # accumulator reads cut the post-unblock activation backlog.
            if not SW_CHUNKS:
                per = NCH_HW // ngrp
                for k in range(NCH_HW):
                    nc.scalar.activation(out=ex_all[:, k * cw:(k + 1) * cw],
                                         in_=ctls[k][:], func=AF.Exp)
                    if (k + 1) % per == 0:
                        g = k // per
                        ln = cp.tile([128, per * cw], F32, tag=f"cl{g}")
                        nc.scalar.activation(
                            out=ln[:], in_=ex_all[:, g * per * cw:(g + 1) * per * cw],
                            func=AF.Ln, bias=1.0, accum_out=acc[:, g:g + 1])
            else:
                for k in range(NJOBS):
                    w = ctls[k].shape[1]
                    ex = cp.tile([128, w], F32, tag=f"ce{k}")
                    nc.scalar.activation(out=ex[:], in_=ctls[k][:], func=AF.Exp)
                    ln = cp.tile([128, w], F32, tag=f"cl{k}")
                    nc.scalar.activation(out=ln[:], in_=ex[:], func=AF.Ln, bias=1.0,
                                         accum_out=acc[:, k:k + 1])

            # ---- outputs.  stats go out early (right after the q=1 matmul);
            # the conf accumulator goes out raw [128, NJOBS] as the very last
            # step (host does the final sum), so the post-window critical
            # path is one sync-queue DMA.  No gpsimd at the end -> the Pool
            # drain is off the critical path.
            so = pp.tile([5, 1], F32)
            nc.vector.tensor_copy(out=so[:], in_=stats_ps[:])
            nc.scalar.dma_start(out=out_d.ap()[0:5, :], in_=so[:])
            # acc DMA on the Scalar stream: it directly follows the last
            # accumulator read there, avoiding a cross-engine sem hop.
            nc.scalar.dma_start(out=acc_d.ap()[:, :], in_=acc[:])
    if split:
        _split_multi_waits(nc)
    return nc


_NC_CACHE = None


def _get_nc():
    global _NC_CACHE
    if _NC_CACHE is None:
        _NC_CACHE = build_nc()
    return _NC_CACHE


def make_in_maps(predictions, targets):
    preds = np.ascontiguousarray(np.asarray(predictions, dtype=np.float32)).reshape(NCORES, ROWS, 85)
    tgts = np.ascontiguousarray(np.asarray(targets, dtype=np.float32)).reshape(NCORES, NT, 5)
    return [{"predictions": preds[c], "targets": tgts[c]} for c in range(NCORES)]


def combine_partials(parts):
    """parts: list of 8 (out[8,1], acc[128,NJOBS]) -> 5-tuple of losses"""
    s = np.sum([p[0].reshape(-1) for p in parts], axis=0, dtype=np.float64)
    spsum = np.float32(np.sum([np.sum(p[1], dtype=np.float64) for p in parts]))
    xy, wh, cls_, nt, corr = [np.float32(v) for v in s[:5]]
    denom = np.float32(max(float(nt), 1.0))
    loss_xy = np.float32(xy / denom)
    loss_wh = np.float32(wh / denom)
    loss_cls = np.float32(cls_ / denom)
    loss_conf = np.float32((spsum + corr) / np.float32(B * HWC))
    total = np.float32(5.0 * loss_xy + 5.0 * loss_wh + loss_conf + loss_cls)
    return total, loss_xy, loss_wh, loss_conf, loss_cls


def kernel(predictions, targets, H=None, W=None):
    from concourse.bass_utils import run_bass_kernel_spmd

    nc = _get_nc()
    in_maps = make_in_maps(predictions, targets)
    res = run_bass_kernel_spmd(nc, in_maps, core_ids=list(range(NCORES)))
    parts = [(res.results[c]["out"], res.results[c]["acc"]) for c in range(NCORES)]
    return combine_partials(parts)


# revision 43
# speedup vs baseline: 1.0139x; 1.0139x over previous
"""Trainium2 Bass kernel for nn_MinimalLoss (YOLO-style detection loss).

Strategy (data-parallel over 8 NeuronCores, 4 batches each):
  The only parts of `predictions` [B, HW, 85] that matter are:
    * column 4 (conf logit) of every cell  -> sum of softplus(x)
      ( -ln(1-sigmoid(x)) = ln(1+e^x) ), computed with Exp+Ln activations
      (both live in the same act-func table set -> one table load total).
    * the <=200 rows per core addressed by targets -> gathered via
      indirect DMA; xy/wh/cls/conf-correction terms computed on-chip.

  The strided conf-channel read (one 4-byte element per 340B row) is the
  problem's bottleneck: 102,400 scattered 64B HBM bursts per core at the
  measured ~1.6-1.8 scattered-reads/ns/core ceiling -> a ~61-64us DMA
  window that everything else (per-target math, dedup, reductions) is
  overlapped under.  Per-core partials (5 stats via a PSUM matmul + the
  raw conf accumulator) are combined on host.
"""
import numpy as np

import concourse.bass as bass
import concourse.mybir as mybir
import concourse.tile as tile
from concourse.bass import IndirectOffsetOnAxis
from concourse.masks import make_identity

F32 = mybir.dt.float32
I32 = mybir.dt.int32
AF = mybir.ActivationFunctionType
ALU = mybir.AluOpType
AX = mybir.AxisListType

B, HWC, C, T = 32, 25600, 80, 50          # full problem
H = W = 160
NCORES = 8
BL = B // NCORES                          # 4 batches per core
ROWS = BL * HWC                           # 102400 prediction rows per core
NT = BL * T                               # 200 targets per core
HALF = NT // 2                            # 100 targets per half (2 batches)
MAGIC = float(np.float32(2 ** 23))

# conf transport: all via the sync HWDGE ring, one 4B descriptor/packet
# per cell, 16 chunks.  The binding resource is the HBM scattered-read
# rate (~1.7-1.8 reads/ns/core; every cell is its own 64B burst), so the
# ~61-64us window is irreducible for this layout.  Measured dead ends:
#   * a second HWDGE queue (scalar): same total rate, and descriptor
#     generation blocks the issuing engine's sequencer (ring
#     backpressure), starving all activations;
#   * SWDGE (gpsimd Q7 gen, ~0.74ns/desc): adding it on top of HWDGE
#     yields only the same shared ~1.8 reads/ns, minus ring-oscillation
#     and per-packet round-robin losses — measured net-negative;
#   * 344B "pair" descriptors (2 cells/run): 172B/cell of engine bus and
#     3 HBM bursts/cell cap them below singles.
import os as _os
SW_COLS = int(_os.environ.get("SW_COLS", "0"))    # experimental SWDGE share
HW_COLS = 800 - SW_COLS
NCH_HW = 16
assert HW_COLS % NCH_HW == 0
if SW_COLS == 0:
    SW_CHUNKS = []
elif SW_COLS >= 64:
    _body = SW_COLS - 32
    _n = max(1, round(_body / 42))
    SW_CHUNKS = [16, 16] + [_body // _n + (1 if i < _body % _n else 0)
                            for i in range(_n)]
else:
    SW_CHUNKS = [SW_COLS]
assert sum(SW_CHUNKS) == SW_COLS
NJOBS = len(SW_CHUNKS) + NCH_HW


def _floor(nc, sb, dst, src, n):
    """dst = floor(src) for 0 <= src < 2^22, exact (round-to-nearest fixup)."""
    r = sb.tile([n, 1], F32, tag="fl_r")
    adj = sb.tile([n, 1], F32, tag="fl_a")
    nc.vector.tensor_scalar_add(r[:], src, MAGIC)
    nc.vector.tensor_scalar_add(r[:], r[:], -MAGIC)
    nc.vector.tensor_tensor(out=adj[:], in0=r[:], in1=src, op=ALU.is_gt)
    nc.vector.tensor_tensor(out=dst, in0=r[:], in1=adj[:], op=ALU.subtract)


def _split_multi_waits(nc):
    """Walrus codegen accepts at most ONE sync wait per instruction; hoist
    extras onto standalone EventSemaphore (wait) ops on the same engine."""
    n = 0
    for func in nc.m.functions:
        for block in func.blocks:
            out = []
            for inst in block.instructions:
                si = inst.sync_info
                if si is not None and si.on_wait and len(si.on_wait) > 1:
                    waits = list(si.on_wait)
                    for w in waits[:-1]:
                        n += 1
                        nop = mybir.InstEventSemaphore(
                            name=f"{inst.name}_sw{n}", engine=inst.engine,
                            ins=[], outs=[])
                        nop.sync_info = mybir.SyncInfo(on_wait=[w], on_update=[])
                        out.append(nop)
                    inst.sync_info = mybir.SyncInfo(on_wait=[waits[-1]],
                                                    on_update=list(si.on_update))
                out.append(inst)
            if n:
                block.instructions[:] = out
    return n


def build_nc(split=True):
    nc = bass.Bass("TRN2", target_bir_lowering=False, debug=False)
    ngrp = 4 if not SW_CHUNKS else NJOBS  # ln/accum groups (4 chunks each)
    pred_d = nc.dram_tensor("predictions", [ROWS, 85], F32, kind="ExternalInput")
    tgt_d = nc.dram_tensor("targets", [NT, 5], F32, kind="ExternalInput")
    out_d = nc.dram_tensor("out", [8, 1], F32, kind="ExternalOutput")
    acc_d = nc.dram_tensor("acc", [128, ngrp], F32, kind="ExternalOutput")

    pred_ap = pred_d.ap()
    conf = pred_ap[:, 4:5].rearrange("(p j) o -> p (j o)", p=128)  # [128, 800]

    with tile.TileContext(nc) as tc:
        with tc.tile_pool(name="persist", bufs=1) as pp, \
             tc.tile_pool(name="conf", bufs=1) as cp, \
             tc.tile_pool(name="sb", bufs=2) as sb, \
             tc.tile_pool(name="ps", bufs=1, space="PSUM") as ps:

            # ---- target loads first on the scalar HWDGE ring (tiny).
            P = HALF
            tts = []
            for q in range(2):
                tt = sb.tile([P, 5], F32, tag="tt")
                nc.scalar.dma_start(out=tt[:], in_=tgt_d.ap()[q * P:(q + 1) * P, :])
                tts.append(tt)

            ctls = [None] * NJOBS

            # HWDGE conf chunks (unique all-live tiles so each DMA needs
            # <=1 sync wait — the DIRECT2D codegen limit).  The first SCQ
            # chunks go on the scalar HWDGE ring: two rings drain ~6%
            # faster than one (engines round-robin per packet), and a
            # small leading share only blocks the Scalar sequencer until
            # ~25us — long before any activation deadline.
            scq = int(_os.environ.get("SCQ", "6"))
            cw = HW_COLS // NCH_HW
            for k in range(NCH_HW):
                tl = cp.tile([128, cw], F32, tag=f"ci{k}")
                eng = nc.scalar if k < scq else nc.sync
                eng.dma_start(out=tl[:], in_=conf[:, k * cw:(k + 1) * cw])
                ctls[k] = tl

            # optional SWDGE conf chunks, cols [HW_COLS, 800)
            sw_off = [HW_COLS]
            for w in SW_CHUNKS:
                sw_off.append(sw_off[-1] + w)

            def emit_sw(i):
                w = SW_CHUNKS[i]
                tl = cp.tile([128, w], F32, tag=f"ci{NCH_HW + i}")
                nc.gpsimd.dma_start(out=tl[:], in_=conf[:, sw_off[i]:sw_off[i] + w])
                ctls[NCH_HW + i] = tl

            for i in range(min(2, len(SW_CHUNKS))):
                emit_sw(i)

            # constants (route matmul operands through DVE so each matmul
            # needs at most ONE sync wait — the S3_LW slot limit)
            ident_g = pp.tile([128, 128], F32)
            make_identity(nc, ident_g[:])
            ident = pp.tile([128, 128], F32)
            nc.vector.tensor_copy(out=ident[:], in_=ident_g[:])
            ones = pp.tile([128, 1], F32)
            nc.vector.memset(ones[:], 1.0)
            iotac = pp.tile([128, C], I32)
            nc.gpsimd.iota(iotac[:], pattern=[[1, C]], base=0, channel_multiplier=0)
            iotaf = pp.tile([128, C], F32)
            nc.vector.tensor_copy(out=iotaf[:], in_=iotac[:])
            iotap = pp.tile([128, 1], I32)
            nc.gpsimd.iota(iotap[:], pattern=[[1, 1]], base=0, channel_multiplier=1)
            pf128 = pp.tile([128, 1], F32)
            nc.vector.tensor_copy(out=pf128[:], in_=iotap[:])
            iotar = pp.tile([128, 128], I32)
            nc.gpsimd.iota(iotar[:], pattern=[[1, 128]], base=0, channel_multiplier=0)
            iotarf = pp.tile([128, 128], F32)
            nc.vector.tensor_copy(out=iotarf[:], in_=iotar[:])
            tri = pp.tile([128, 128], F32)  # tri[p, f] = 1.0 iff f < p
            nc.vector.tensor_tensor(out=tri[:], in0=pf128[:].to_broadcast([128, 128]),
                                    in1=iotarf[:], op=ALU.is_gt)

            acc = pp.tile([128, ngrp], F32)
            ex_all = pp.tile([128, HW_COLS], F32)

            # ---- per-target phase: two halves of 100 targets (2 whole
            # batches each).  Runs entirely under the conf-DMA window.
            stats_ps = ps.tile([5, 1], F32, space="PSUM")
            for q in range(2):
                tt = tts[q]

                xW = sb.tile([P, 1], F32, tag="xW")
                yH = sb.tile([P, 1], F32, tag="yH")
                nc.vector.tensor_scalar_mul(xW[:], tt[:, 1:2], float(W))
                nc.vector.tensor_scalar_mul(yH[:], tt[:, 2:3], float(H))
                gx = sb.tile([P, 1], F32, tag="gx")
                gy = sb.tile([P, 1], F32, tag="gy")
                _floor(nc, sb, gx[:], xW[:], P)
                _floor(nc, sb, gy[:], yH[:], P)

                # validity
                vf = sb.tile([P, 1], F32, tag="vf")
                tmp = sb.tile([P, 1], F32, tag="tmp")
                nc.vector.tensor_scalar(out=vf[:], in0=gx[:], scalar1=0.0, scalar2=None, op0=ALU.is_ge)
                nc.vector.tensor_scalar(out=tmp[:], in0=gx[:], scalar1=float(W), scalar2=None, op0=ALU.is_lt)
                nc.vector.tensor_tensor(out=vf[:], in0=vf[:], in1=tmp[:], op=ALU.mult)
                nc.vector.tensor_scalar(out=tmp[:], in0=gy[:], scalar1=0.0, scalar2=None, op0=ALU.is_ge)
                nc.vector.tensor_tensor(out=vf[:], in0=vf[:], in1=tmp[:], op=ALU.mult)
                nc.vector.tensor_scalar(out=tmp[:], in0=gy[:], scalar1=float(H), scalar2=None, op0=ALU.is_lt)
                nc.vector.tensor_tensor(out=vf[:], in0=vf[:], in1=tmp[:], op=ALU.mult)

                # cell + per-core row index
                gxi = sb.tile([P, 1], F32, tag="gxi")
                gyi = sb.tile([P, 1], F32, tag="gyi")
                nc.vector.tensor_scalar(out=gxi[:], in0=gx[:], scalar1=0.0, scalar2=float(W - 1),
                                        op0=ALU.max, op1=ALU.min)
                nc.vector.tensor_scalar(out=gyi[:], in0=gy[:], scalar1=0.0, scalar2=float(H - 1),
                                        op0=ALU.max, op1=ALU.min)
                cell = sb.tile([P, 1], F32, tag="cell")
                nc.vector.tensor_scalar_mul(cell[:], gyi[:], float(W))
                nc.vector.tensor_tensor(out=cell[:], in0=cell[:], in1=gxi[:], op=ALU.add)

                rowf = sb.tile([P, 1], F32, tag="rowf")
                # batch offset: (2q + (t>=50)) * HWC
                nc.vector.tensor_scalar(out=rowf[:], in0=pf128[:P, :], scalar1=float(T), scalar2=None,
                                        op0=ALU.is_ge)
                nc.vector.tensor_scalar(out=rowf[:], in0=rowf[:], scalar1=float(HWC),
                                        scalar2=float(2 * q * HWC), op0=ALU.mult, op1=ALU.add)
                nc.vector.tensor_tensor(out=rowf[:], in0=rowf[:], in1=cell[:], op=ALU.add)
                idx = sb.tile([P, 1], I32, tag="idx")
                nc.vector.tensor_copy(out=idx[:], in_=rowf[:])

                # dedup key: valid -> rowf ; invalid -> unique negative
                negk = sb.tile([P, 1], F32, tag="negk")
                nc.vector.tensor_scalar(out=negk[:], in0=pf128[:P, :], scalar1=-1.0,
                                        scalar2=-(1.0 + 100.0 * q), op0=ALU.mult, op1=ALU.add)
                key = sb.tile([P, 1], F32, tag="key")
                nc.vector.tensor_tensor(out=key[:], in0=rowf[:], in1=negk[:], op=ALU.subtract)
                nc.vector.tensor_tensor(out=key[:], in0=key[:], in1=vf[:], op=ALU.mult)
                nc.vector.tensor_tensor(out=key[:], in0=key[:], in1=negk[:], op=ALU.add)

                # gather prediction rows
                rows = sb.tile([P, 85], F32, tag="rows")
                nc.gpsimd.indirect_dma_start(
                    out=rows[:], out_offset=None, in_=pred_ap[:, :],
                    in_offset=IndirectOffsetOnAxis(ap=idx[:, :1], axis=0))

                # A  = ln(1+e^x)  = -ln(1-sigmoid(x))   (clamp never fires:
                # B  = ln(1+e^-x) = -ln(sigmoid(x))      |x| <~ 0.6 here)
                ep = sb.tile([P, 85], F32, tag="ep")
                nc.scalar.activation(out=ep[:], in_=rows[:], func=AF.Exp)
                en = sb.tile([P, 85], F32, tag="en")
                nc.scalar.activation(out=en[:], in_=rows[:], func=AF.Exp, scale=-1.0)
                At = sb.tile([P, 85], F32, tag="At")
                nc.scalar.activation(out=At[:], in_=ep[:], func=AF.Ln, bias=1.0)
                Bt = sb.tile([P, 85], F32, tag="Bt")
                nc.scalar.activation(out=Bt[:], in_=en[:], func=AF.Ln, bias=1.0)

                # per_cls = (1/C) * sum_c [ A + onehot*(B - A) ]
                oh = sb.tile([P, C], F32, tag="oh")
                nc.vector.tensor_tensor(out=oh[:], in0=iotaf[:P, :],
                                        in1=tt[:, 0:1].to_broadcast([P, C]), op=ALU.is_equal)
                dlt = sb.tile([P, C], F32, tag="dlt")
                nc.vector.tensor_tensor(out=dlt[:], in0=Bt[:, 5:85], in1=At[:, 5:85], op=ALU.subtract)
                nc.vector.tensor_tensor(out=dlt[:], in0=dlt[:], in1=oh[:], op=ALU.mult)
                nc.vector.tensor_tensor(out=dlt[:], in0=dlt[:], in1=At[:, 5:85], op=ALU.add)
                pcls = sb.tile([P, 1], F32, tag="pcls")
                nc.vector.reduce_sum(out=pcls[:], in_=dlt[:], axis=AX.X)
                nc.vector.tensor_scalar_mul(pcls[:], pcls[:], 1.0 / C)

                # conf correction term: ct = lnn - lnp = B[4] - A[4]
                ct = sb.tile([P, 1], F32, tag="ct")
                nc.vector.tensor_tensor(out=ct[:], in0=Bt[:, 4:5], in1=At[:, 4:5], op=ALU.subtract)

                # per_xy: sigmoid(x) = 1/(1+e^-x)
                sxy = sb.tile([P, 2], F32, tag="sxy")
                nc.vector.tensor_scalar_add(sxy[:], en[:, 0:2], 1.0)
                nc.vector.reciprocal(out=sxy[:], in_=sxy[:])
                txy = sb.tile([P, 2], F32, tag="txy")
                nc.vector.tensor_tensor(out=txy[:, 0:1], in0=xW[:], in1=gx[:], op=ALU.subtract)
                nc.vector.tensor_tensor(out=txy[:, 1:2], in0=yH[:], in1=gy[:], op=ALU.subtract)
                dxy = sb.tile([P, 2], F32, tag="dxy")
                nc.vector.tensor_tensor(out=dxy[:], in0=sxy[:], in1=txy[:], op=ALU.subtract)
                nc.vector.tensor_tensor(out=dxy[:], in0=dxy[:], in1=dxy[:], op=ALU.mult)
                pxy = sb.tile([P, 1], F32, tag="pxy")
                nc.vector.reduce_sum(out=pxy[:], in_=dxy[:], axis=AX.X)
                nc.vector.tensor_scalar_mul(pxy[:], pxy[:], 0.5)

                # per_wh: pred_wh = e^x (already have ep)
                twh = sb.tile([P, 2], F32, tag="twh")
                nc.vector.tensor_scalar_mul(twh[:, 0:1], tt[:, 3:4], float(W))
                nc.vector.tensor_scalar_mul(twh[:, 1:2], tt[:, 4:5], float(H))
                dwh = sb.tile([P, 2], F32, tag="dwh")
                nc.vector.tensor_tensor(out=dwh[:], in0=ep[:, 2:4], in1=twh[:], op=ALU.subtract)
                nc.vector.tensor_tensor(out=dwh[:], in0=dwh[:], in1=dwh[:], op=ALU.mult)
                pwh = sb.tile([P, 1], F32, tag="pwh1")
                nc.vector.reduce_sum(out=pwh[:], in_=dwh[:], axis=AX.X)
                nc.vector.tensor_scalar_mul(pwh[:], pwh[:], 0.5)

                # dedup: first-occurrence weight w
                keyT_ps = ps.tile([P, P], F32, space="PSUM", tag="keyT_ps")
                nc.tensor.transpose(out=keyT_ps[:], in_=key[:].to_broadcast([P, P]),
                                    identity=ident[:P, :P])
                keyT = sb.tile([P, P], F32, tag="keyT")
                nc.vector.tensor_copy(out=keyT[:], in_=keyT_ps[:])
                eq = sb.tile([P, P], F32, tag="eq")
                nc.vector.tensor_tensor(out=eq[:], in0=key[:].to_broadcast([P, P]),
                                        in1=keyT[:], op=ALU.is_equal)
                nc.vector.tensor_tensor(out=eq[:], in0=eq[:], in1=tri[:P, :P], op=ALU.mult)
                dup = sb.tile([P, 1], F32, tag="dup")
                nc.vector.reduce_max(out=dup[:], in_=eq[:], axis=AX.X)
                wfo = sb.tile([P, 1], F32, tag="wfo")
                nc.vector.tensor_scalar(out=wfo[:], in0=dup[:], scalar1=-1.0, scalar2=1.0,
                                        op0=ALU.mult, op1=ALU.add)
                nc.vector.tensor_tensor(out=wfo[:], in0=wfo[:], in1=vf[:], op=ALU.mult)

                # stats columns: vf*pxy, vf*pwh, vf*pcls, vf, w*ct
                stats = sb.tile([P, 5], F32, tag="stats")
                nc.vector.tensor_tensor(out=stats[:, 0:1], in0=pxy[:], in1=vf[:], op=ALU.mult)
                nc.vector.tensor_tensor(out=stats[:, 1:2], in0=pwh[:], in1=vf[:], op=ALU.mult)
                nc.vector.tensor_tensor(out=stats[:, 2:3], in0=pcls[:], in1=vf[:], op=ALU.mult)
                nc.vector.tensor_copy(out=stats[:, 3:4], in_=vf[:])
                nc.vector.tensor_tensor(out=stats[:, 4:5], in0=ct[:], in1=wfo[:], op=ALU.mult)

                nc.tensor.matmul(out=stats_ps[:], lhsT=stats[:], rhs=ones[:P, :],
                                 start=(q == 0), stop=(q == 1))

            # remaining SWDGE conf chunks (after the gathers in Pool order)
            for i in range(2, len(SW_CHUNKS)):
                emit_sw(i)

            # ---- conf channel: sum softplus(x) = ln(1 + e^x).
            # One Exp per chunk (pipelines with its DMA), but only one
            # Ln+accum per 4-chunk group: fewer Scalar ops and fewer
            # accumulator reads cut the post-unblock activation backlog.
            if not SW_CHUNKS:
                per = NCH_HW // ngrp
                for k in range(NCH_HW):
                    nc.scalar.activation(out=ex_all[:, k * cw:(k + 1) * cw],
                                         in_=ctls[k][:], func=AF.Exp)
                    if (k + 1) % per == 0:
                        g = k // per
                        ln = cp.tile([128, per * cw], F32, tag=f"cl{g}")
                        nc.scalar.activation(
                            out=ln[:], in_=ex_all[:, g * per * cw:(g + 1) * per * cw],
                            func=AF.Ln, bias=1.0, accum_out=acc[:, g:g + 1])
            else:
                for k in range(NJOBS):
                    w = ctls[k].shape[1]
                    ex = cp.tile([128, w], F32, tag=f"ce{k}")
                    nc.scalar.activation(out=ex[:], in_=ctls[k][:], func=AF.Exp)
                    ln = cp.tile([128, w], F32, tag=f"cl{k}")
                    nc.scalar.activation(out=ln[:], in_=ex[:], func=AF.Ln, bias=1.0,
                                         accum_out=acc[:, k:k + 1])

            # ---- outputs.  stats go out early (right after the q=1 matmul);
            # the conf accumulator goes out raw [128, NJOBS] as the very last
            # step (host does the final sum), so the post-window critical
            # path is one sync-queue DMA.  No gpsimd at the end -> the Pool
            # drain is off the critical path.
            so = pp.tile([5, 1], F32)
            nc.vector.tensor_copy(out=so[:], in_=stats_ps[:])
            nc.scalar.dma_start(out=out_d.ap()[0:5, :], in_=so[:])
            # acc DMA on the Scalar stream: it directly follows the last
            # accumulator read there, avoiding a cross-engine sem hop.
            nc.scalar.dma_start(out=acc_d.ap()[:, :], in_=acc[:])
    if split:
        _split_multi_waits(nc)
    return nc


_NC_CACHE = None


def _get_nc():
    global _NC_CACHE
    if _NC_CACHE is None:
        _NC_CACHE = build_nc()
    return _NC_CACHE


def make_in_maps(predictions, targets):
    preds = np.ascontiguousarray(np.asarray(predictions, dtype=np.float32)).reshape(NCORES, ROWS, 85)
    tgts = np.ascontiguousarray(np.asarray(targets, dtype=np.float32)).reshape(NCORES, NT, 5)
    return [{"predictions": preds[c], "targets": tgts[c]} for c in range(NCORES)]


def combine_partials(parts):
    """parts: list of 8 (out[8,1], acc[128,NJOBS]) -> 5-tuple of losses"""
    s = np.sum([p[0].reshape(-1) for p in parts], axis=0, dtype=np.float64)
    spsum = np.float32(np.sum([np.sum(p[1], dtype=np.float64) for p in parts]))
    xy, wh, cls_, nt, corr = [np.float32(v) for v in s[:5]]
    denom = np.float32(max(float(nt), 1.0))
    loss_xy = np.float32(xy / denom)
    loss_wh = np.float32(wh / denom)
    loss_cls = np.float32(cls_ / denom)
    loss_conf = np.float32((spsum + corr) / np.float32(B * HWC))
    total = np.float32(5.0 * loss_xy + 5.0 * loss_wh + loss_conf + loss_cls)
    return total, loss_xy, loss_wh, loss_conf, loss_cls


def kernel(predictions, targets, H=None, W=None):
    from concourse.bass_utils import run_bass_kernel_spmd

    nc = _get_nc()
    in_maps = make_in_maps(predictions, targets)
    res = run_bass_kernel_spmd(nc, in_maps, core_ids=list(range(NCORES)))
    parts = [(res.results[c]["out"], res.results[c]["acc"]) for c in range(NCORES)]
    return combine_partials(parts)
